# revision 1
# baseline (speedup 1.0000x reference)
"""BertSelfAttention TRN2 kernel.

Problem: B=4, S=2048, H=768, NH=12, HD=64, fp32.
Sharding: 8 cores; core c owns batch b = c//2 and head-group g = c%2
(6 heads = 384 hidden cols). Each core runs the same Bass program on its
shard; host reassembles.

Per-core algorithm (all fp32):
  xT = x^T (PE transpose)                     [768, 2048]
  WT = W^T for q/k/v (PE transpose)           [768, 384]
  QT/KT = W @ xT + b   (pair layout: partitions [headA d | headB d])
  V = x @ WvT + bv, scaled by w_k = exp(mask_k), plus a w column
      (folds the additive attention mask into multiplicative row
       weights: softmax_k(s+m) = exp(s)w / sum_k exp(s)w)
  per (pair, q-quarter, kt-pair):
     scoresT[k,q] = K @ QT      (two heads row-packed on the PE, K=64 each)
     E = exp(SCALE * scoresT)   (ScalarE from PSUM, 1024-wide chunks)
     ctxT[d,q] and denom[q] accumulate in PSUM via v_aug = [v*w | w] (M=65)
  final: PE-transpose ctxT -> [q, d] and 1/denom -> [q, 1], multiply, DMA.
"""

import numpy as np

B, S, H = 4, 2048, 768
NH, HD = 12, 64
SCALE = 1.0 / np.sqrt(np.float32(HD)).astype(np.float32)
HPC = H // 2          # 384 hidden cols per core (6 heads)
NHEADS = 6            # heads per core
NPAIR = 3             # head pairs per core
NST = S // 128        # 16 S-tiles
NHT = H // 128        # 6 hidden tiles
NCORES = 8

_CACHE = {}


def build_nc(reps=1, loop_part="all"):
    import concourse.bacc as bacc
    import concourse.mybir as mybir
    import concourse.tile as tile
    from concourse.masks import make_identity

    f32 = mybir.dt.float32
    f32r = mybir.dt.float32r
    bf16 = mybir.dt.bfloat16
    AF = mybir.ActivationFunctionType
    OP = mybir.AluOpType

    nc = bacc.Bacc("TRN2", target_bir_lowering=False, debug=False,
                   num_devices=NCORES)

    x_d = nc.declare_dram_parameter("x", [S, H], f32, isOutput=False)
    wq_d = nc.declare_dram_parameter("wq", [HPC, H], f32, isOutput=False)
    wk_d = nc.declare_dram_parameter("wk", [HPC, H], f32, isOutput=False)
    wv_d = nc.declare_dram_parameter("wv", [HPC, H], f32, isOutput=False)
    bq_d = nc.declare_dram_parameter("bq", [HPC], f32, isOutput=False)
    bk_d = nc.declare_dram_parameter("bk", [HPC], f32, isOutput=False)
    bv_d = nc.declare_dram_parameter("bv", [HPC], f32, isOutput=False)
    mask_d = nc.declare_dram_parameter("mask", [S], f32, isOutput=False)
    out_d = nc.declare_dram_parameter("out", [S, HPC], f32, isOutput=True)

    with tile.TileContext(nc) as tc:
        import contextlib
        with contextlib.ExitStack() as stack:
            consts = stack.enter_context(tc.tile_pool(name="consts", bufs=1))
            p_qk = stack.enter_context(tc.tile_pool(name="p_qk", bufs=1))
            p_v = stack.enter_context(tc.tile_pool(name="p_v", bufs=1))
            p_e = stack.enter_context(tc.tile_pool(name="p_e", bufs=3))
            p_post = stack.enter_context(tc.tile_pool(name="p_post", bufs=1))
            p_rec = stack.enter_context(tc.tile_pool(name="p_rec", bufs=4))

            # ---- constants ----
            ident = consts.tile([128, 128], f32)
            make_identity(nc, ident)
            ones_row = consts.tile([1, 128], f32)
            nc.gpsimd.memset(ones_row, 1.0)
            ones6 = consts.tile([128, NHEADS], f32)
            nc.gpsimd.memset(ones6, 1.0)
            bq_sb = consts.tile([128, NPAIR], f32)
            nc.gpsimd.dma_start(out=bq_sb, in_=bq_d[:].rearrange("(t p) -> p t", p=128))
            bk_sb = consts.tile([128, NPAIR], f32)
            nc.gpsimd.dma_start(out=bk_sb, in_=bk_d[:].rearrange("(t p) -> p t", p=128))
            bv_sb = consts.tile([1, HPC], f32)
            nc.gpsimd.dma_start(out=bv_sb, in_=bv_d[:].rearrange("(o s) -> o s", o=1))
            mask_sb = consts.tile([128, NST], f32)
            nc.gpsimd.dma_start(out=mask_sb, in_=mask_d[:].rearrange("(t p) -> p t", p=128))
            wmask = consts.tile([128, NST], f32)
            nc.scalar.activation(wmask, mask_sb, AF.Exp)

            # ---- persistent ----
            qt = p_qk.tile([128, NPAIR, S], bf16)      # Q^T pairs
            kt_sb = p_qk.tile([128, NPAIR, S], bf16)   # K^T pairs
            v_sb = p_v.tile([128, NST, NHEADS, HD + 1], f32r)
            out_sb = p_post.tile([128, NST, HPC], f32)

            def build():
                with tc.tile_pool(name="p_xtw", bufs=1) as px:
                    xt = px.tile([128, NHT, S], f32r)
                    wt_q = px.tile([128, NHT, HPC], f32r)
                    wt_k = px.tile([128, NHT, HPC], f32r)
                    wt_v = px.tile([128, NHT, HPC], f32r)

                    # -- transposes of x and W (own psum + transient pool) --
                    with tc.tile_pool(name="p_nat", bufs=1) as pn, \
                            tc.tile_pool(name="psum_util", bufs=2, space="PSUM") as pu:
                        for stq in range(4):
                            xtiles = []
                            for c in range(4):
                                st = stq * 4 + c
                                x_nat = pn.tile([128, H], f32, tag="xnat", bufs=6,
                                                name="x_nat")
                                nc.gpsimd.dma_start(out=x_nat, in_=x_d[st * 128:(st + 1) * 128, :])
                                xtiles.append(x_nat)
                            for ht in range(NHT):
                                pxm = pu.tile([128, 512], f32, tag="tp1", name="pxm")
                                for c in range(4):
                                    nc.tensor.transpose(
                                        pxm[:, c * 128:(c + 1) * 128],
                                        xtiles[c][:, ht * 128:(ht + 1) * 128],
                                        ident)
                                nc.vector.tensor_copy(xt[:, ht, stq * 512:(stq + 1) * 512], pxm)
                        for wd, wt in ((wq_d, wt_q), (wk_d, wt_k), (wv_d, wt_v)):
                            wtiles = []
                            for ot in range(3):
                                w_nat = pn.tile([128, H], f32, tag="wnat", bufs=3,
                                                name="w_nat")
                                nc.gpsimd.dma_start(out=w_nat, in_=wd[ot * 128:(ot + 1) * 128, :])
                                wtiles.append(w_nat)
                            for ht in range(NHT):
                                pw = pu.tile([128, HPC], f32, tag="tpw", name="pw",
                                             bufs=1)
                                for ot in range(3):
                                    nc.tensor.transpose(
                                        pw[:, ot * 128:(ot + 1) * 128],
                                        wtiles[ot][:, ht * 128:(ht + 1) * 128],
                                        ident)
                                nc.vector.tensor_copy(wt[:, ht, :], pw)

                    with tc.tile_pool(name="psum_qkv", bufs=1, space="PSUM") as psum_qkv:
                        def emit_qk(pair):
                            for wt, dst, bias in ((wt_q, qt, bq_sb), (wt_k, kt_sb, bk_sb)):
                                for sc in range(4):
                                    pq = psum_qkv.tile([128, 512], f32, tag="qt",
                                                       name="pq", bufs=2)
                                    for ht in range(NHT):
                                        nc.tensor.matmul(
                                            pq,
                                            lhsT=wt[:, ht, pair * 128:(pair + 1) * 128],
                                            rhs=xt[:, ht, sc * 512:(sc + 1) * 512],
                                            start=(ht == 0), stop=(ht == NHT - 1))
                                    nc.vector.tensor_scalar(
                                        out=dst[:, pair, sc * 512:(sc + 1) * 512],
                                        in0=pq, scalar1=bias[:, pair:pair + 1],
                                        scalar2=None, op0=OP.add)

                        emit_qk(0)
                        for st in range(NST):
                            pv = psum_qkv.tile([128, HPC], f32, tag="v", name="pv", bufs=2)
                            for ht in range(NHT):
                                nc.tensor.matmul(
                                    pv,
                                    lhsT=xt[:, ht, st * 128:(st + 1) * 128],
                                    rhs=wt_v[:, ht, :],
                                    start=(ht == 0), stop=False)
                            nc.tensor.matmul(pv, lhsT=ones_row, rhs=bv_sb,
                                             start=False, stop=True)
                            nc.vector.tensor_scalar(
                                out=v_sb[:, st, :, 0:HD],
                                in0=pv.rearrange("p (h d) -> p h d", h=NHEADS),
                                scalar1=wmask[:, st:st + 1], scalar2=None, op0=OP.mult)
                            nc.vector.tensor_scalar(
                                out=v_sb[:, st, :, HD], in0=ones6,
                                scalar1=wmask[:, st:st + 1], scalar2=None, op0=OP.mult)
                        emit_qk(1)
                        emit_qk(2)

                with tc.tile_pool(name="p_ctx", bufs=1) as p_ctx:
                    ctx_sa = p_ctx.tile([65, NPAIR, 4, 512], f32)
                    ctx_sb2 = p_ctx.tile([65, NPAIR, 4, 512], f32)
                    with tc.tile_pool(name="psum_attn", bufs=1, space="PSUM") as pa:
                        def attn(pair):
                            """Attention for one head pair + inline epilogue."""
                            for qtr in range(4):
                                qs = qtr * 512
                                ctx_a = pa.tile([65, 512], f32, tag="ctxa",
                                                name="ctx_a")
                                ctx_b = pa.tile([65, 512], f32, tag="ctxb",
                                                name="ctx_b")
                                for ktp in range(8):
                                    sa = pa.tile([128, 1024], f32, tag="s",
                                                 bufs=3, name="sa")
                                    sb_ = pa.tile([128, 1024], f32, tag="s",
                                                  bufs=3, name="sb_")
                                    for par in range(2):
                                        kt = 2 * ktp + par
                                        nc.tensor.matmul(
                                            sa[:, par * 512:(par + 1) * 512],
                                            lhsT=kt_sb[0:64, pair, kt * 128:(kt + 1) * 128],
                                            rhs=qt[0:64, pair, qs:qs + 512],
                                            start=True, stop=True)
                                        nc.tensor.matmul(
                                            sb_[:, par * 512:(par + 1) * 512],
                                            lhsT=kt_sb[64:128, pair, kt * 128:(kt + 1) * 128],
                                            rhs=qt[64:128, pair, qs:qs + 512],
                                            start=True, stop=True)
                                    e_a = p_e.tile([128, 1024], f32r, tag="ea",
                                                   name="e_a")
                                    e_b = p_e.tile([128, 1024], f32r, tag="eb",
                                                   name="e_b")
                                    nc.scalar.activation(e_a, sa, AF.Exp,
                                                         scale=float(SCALE))
                                    nc.scalar.activation(e_b, sb_, AF.Exp,
                                                         scale=float(SCALE))
                                    for par in range(2):
                                        kt = 2 * ktp + par
                                        first = (ktp == 0 and par == 0)
                                        last = (ktp == 7 and par == 1)
                                        nc.tensor.matmul(
                                            ctx_a,
                                            lhsT=v_sb[:, kt, 2 * pair, :],
                                            rhs=e_a[:, par * 512:(par + 1) * 512],
                                            start=first, stop=last)
                                        nc.tensor.matmul(
                                            ctx_b,
                                            lhsT=v_sb[:, kt, 2 * pair + 1, :],
                                            rhs=e_b[:, par * 512:(par + 1) * 512],
                                            start=first, stop=last)
                                nc.vector.tensor_copy(
                                    ctx_sa[0:64, pair, qtr, :], ctx_a[0:64, :])
                                nc.vector.tensor_copy(
                                    ctx_sa[64:65, pair, qtr, :], ctx_a[64:65, :])
                                nc.vector.tensor_copy(
                                    ctx_sb2[0:64, pair, qtr, :], ctx_b[0:64, :])
                                nc.vector.tensor_copy(
                                    ctx_sb2[64:65, pair, qtr, :], ctx_b[64:65, :])

                        attn(0)
                        attn(1)
                        attn(2)


                    # ---- phase 3: transpose back, normalize, store ----
                    with tc.tile_pool(name="ppost", bufs=2, space="PSUM") as pp:
                        for qtr in range(4):
                            for j in range(4):
                                st = qtr * 4 + j
                                for hl in range(NHEADS):
                                    pair, odd = hl // 2, hl % 2
                                    blk = ctx_sb2 if odd else ctx_sa
                                    tp = pp.tile([128, HD], f32,
                                                 tag="tpa" if hl % 2 == 0 else "tpb",
                                                 name="tp")
                                    nc.tensor.transpose(
                                        tp,
                                        blk[0:64, pair, qtr, j * 128:(j + 1) * 128],
                                        ident[0:64, 0:64])
                                    tpd = pp.tile([128, 1], f32,
                                                  tag="tpda" if hl % 2 == 0 else "tpdb",
                                                  name="tpd")
                                    nc.tensor.transpose(
                                        tpd,
                                        blk[64:65, pair, qtr, j * 128:(j + 1) * 128],
                                        ident[64:65, 64:65])
                                    rec = p_rec.tile([128, 1], f32, tag="rec", name="rec")
                                    nc.vector.reciprocal(rec, tpd)
                                    nc.vector.tensor_scalar(
                                        out=out_sb[:, st, hl * HD:(hl + 1) * HD],
                                        in0=tp, scalar1=rec,
                                        scalar2=None, op0=OP.mult)
                                nc.gpsimd.dma_start(
                                    out=out_d[st * 128:(st + 1) * 128, :],
                                    in_=out_sb[:, st, :])


            if reps == 1:
                build()
            else:
                with tc.For_i(0, reps, 1):
                    build()

    nc.compile()
    return nc


def make_runner(nc):
    """jit-compiled shard_map runner over 8 cores."""
    import jax
    import numpy as _np
    from jax.sharding import Mesh, NamedSharding, PartitionSpec
    from jax.experimental.shard_map import shard_map
    import concourse.mybir as mybir
    from concourse.bass2jax import (_bass_exec_p, install_neuronx_cc_hook,
                                    partition_id_tensor)

    install_neuronx_cc_hook()
    part_name = nc.partition_id_tensor.name if nc.partition_id_tensor else None
    in_names, out_names, out_avals, out_shapes = [], [], [], []
    for alloc in nc.m.functions[0].allocations:
        if not isinstance(alloc, mybir.MemoryLocationSet):
            continue
        name = alloc.memorylocations[0].name
        if alloc.kind == "ExternalInput":
            if name != part_name:
                in_names.append(name)
        elif alloc.kind == "ExternalOutput":
            out_names.append(name)
            shape = tuple(alloc.tensor_shape)
            dtype = mybir.dt.np(alloc.dtype)
            out_avals.append(jax.core.ShapedArray(shape, dtype))
            out_shapes.append((shape, dtype))
    n_params = len(in_names)
    all_in_names = list(in_names) + list(out_names)
    if part_name is not None:
        all_in_names.append(part_name)

    def _body(*args):
        operands = list(args)
        if part_name is not None:
            operands.append(partition_id_tensor())
        outs = _bass_exec_p.bind(
            *operands,
            out_avals=tuple(out_avals),
            in_names=tuple(all_in_names),
            out_names=tuple(out_names),
            lowering_input_output_aliases=(),
            sim_require_finite=True,
            sim_require_nnan=True,
            nc=nc,
        )
        return tuple(outs)

    devices = jax.devices()[:NCORES]
    mesh = Mesh(_np.asarray(devices), ("core",))
    sharded = jax.jit(
        shard_map(_body, mesh=mesh,
                  in_specs=(PartitionSpec("core"),) * (n_params + len(out_names)),
                  out_specs=(PartitionSpec("core"),) * len(out_names),
                  check_rep=False),
        keep_unused=True)
    sh = NamedSharding(mesh, PartitionSpec("core"))

    def stage(in_maps):
        import jax as _jax
        concat_in = [
            _jax.device_put(
                _np.ascontiguousarray(
                    _np.concatenate([_np.asarray(m[name]) for m in in_maps], axis=0)),
                sh)
            for name in in_names]
        concat_zeros = [
            _jax.device_put(_np.zeros((NCORES * sh_[0], *sh_[1:]), dt), sh)
            for (sh_, dt) in out_shapes]
        return concat_in, concat_zeros

    def run(concat_in, concat_zeros):
        import jax as _jax
        outs = sharded(*concat_in, *concat_zeros)
        _jax.block_until_ready(outs)
        return outs

    def unpack(outs):
        res = []
        for c in range(NCORES):
            m = {}
            for i, name in enumerate(out_names):
                shape, dt = out_shapes[i]
                m[name] = np.asarray(outs[i]).reshape(NCORES, *shape)[c]
            res.append(m)
        return res

    return stage, run, unpack


def shard_inputs(hidden_states, attention_mask, Wq, bq, Wk, bk, Wv, bv):
    hs = np.asarray(hidden_states, dtype=np.float32)
    am = np.asarray(attention_mask, dtype=np.float32)
    Wq, Wk, Wv = (np.asarray(w, dtype=np.float32) for w in (Wq, Wk, Wv))
    bq, bk, bv = (np.asarray(b, dtype=np.float32) for b in (bq, bk, bv))
    in_maps = []
    for c in range(NCORES):
        b = c // 2
        g = c % 2
        rows = slice(g * HPC, (g + 1) * HPC)
        in_maps.append({
            "x": np.ascontiguousarray(hs[b]),
            "wq": np.ascontiguousarray(Wq[rows]),
            "wk": np.ascontiguousarray(Wk[rows]),
            "wv": np.ascontiguousarray(Wv[rows]),
            "bq": np.ascontiguousarray(bq[rows]),
            "bk": np.ascontiguousarray(bk[rows]),
            "bv": np.ascontiguousarray(bv[rows]),
            "mask": np.ascontiguousarray(am[b, 0, 0, :]),
        })
    return in_maps


def unshard_outputs(results):
    out = np.empty((B, S, H), dtype=np.float32)
    for c in range(NCORES):
        b = c // 2
        g = c % 2
        out[b][:, g * HPC:(g + 1) * HPC] = results[c]["out"]
    return out


def get_compiled(reps=1, loop_part="all"):
    key = (reps, loop_part)
    if key in _CACHE:
        return _CACHE[key]
    if True:
        nc = build_nc(reps, loop_part)
        stage, run, unpack = make_runner(nc)
        _CACHE[key] = (nc, stage, run, unpack)
    return _CACHE[key]


def kernel(hidden_states, attention_mask, Wq, bq, Wk, bk, Wv, bv):
    _, stage, run, unpack = get_compiled(reps=1)
    in_maps = shard_inputs(hidden_states, attention_mask, Wq, bq, Wk, bk, Wv, bv)
    ci, cz = stage(in_maps)
    outs = run(ci, cz)
    return unshard_outputs(unpack(outs))



# revision 3
# speedup vs baseline: 1.1158x; 1.1158x over previous
"""BertSelfAttention TRN2 kernel (v2 — ACT-engine-saturated design).

Problem: B=4, S=2048, H=768, NH=12, HD=64, fp32.
Sharding: 8 cores; core c owns batch b = c//2 and head-group g = c%2
(6 heads = 384 hidden cols). Each core runs the same Bass program on its
shard; host reassembles.

Bottleneck analysis: the softmax exp is 6*S^2 = 25.2M elements per core
and can only run on the Activation engine (1 elem/cycle/partition
@1.2GHz) => ~199us floor. Everything else is organized to hide under
that: all matmuls in bf16 (1 col/cycle on PE), context accumulated in
natural [q, d] layout with a fused denominator column (no output
transposes), and the emission order software-pipelines scores(k+1)
between the two ctx halves of k so ACT never waits on PSUM buffers.

Per-core algorithm (fp32 PSUM, bf16 operands):
  xt = x^T, wt = W^T          (PE transposes of bf16 casts)
  QT/KT[pair] = W @ xT + b    (partitions = [headA d | headB d])
  V = x @ WvT + bv, scaled by w_k = exp(mask_k); augmented col w
  per (qtr, pair, ktp):
     scoresT[k, q] = K @ QT   (2 heads x 2 k-tiles, N=512)
     E = exp(SCALE * scoresT) (ACT, [128,1024] from PSUM -> bf16 SBUF)
     ctx[q, 65] += E^T @ v_aug  (natural layout, denom in col 64)
  epilogue: rec = 1/ctx[:,64]; out = ctx[:,0:64] * rec  (DVE only)
"""

import numpy as np

B, S, H = 4, 2048, 768
NH, HD = 12, 64
SCALE = 1.0 / np.sqrt(np.float32(HD)).astype(np.float32)
HPC = H // 2          # 384 hidden cols per core (6 heads)
NHEADS = 6            # heads per core
NPAIR = 3             # head pairs per core
NST = S // 128        # 16 S-tiles
NHT = H // 128        # 6 hidden tiles
NCORES = 8

_CACHE = {}


def build_nc(reps=1, loop_part="all"):
    import concourse.bacc as bacc
    import concourse.mybir as mybir
    import concourse.tile as tile
    from concourse.masks import make_identity

    f32 = mybir.dt.float32
    bf16 = mybir.dt.bfloat16
    AF = mybir.ActivationFunctionType
    OP = mybir.AluOpType

    nc = bacc.Bacc("TRN2", target_bir_lowering=False, debug=False,
                   num_devices=NCORES)

    x_d = nc.declare_dram_parameter("x", [S, H], f32, isOutput=False)
    wq_d = nc.declare_dram_parameter("wq", [HPC, H], f32, isOutput=False)
    wk_d = nc.declare_dram_parameter("wk", [HPC, H], f32, isOutput=False)
    wv_d = nc.declare_dram_parameter("wv", [HPC, H], f32, isOutput=False)
    bq_d = nc.declare_dram_parameter("bq", [HPC], f32, isOutput=False)
    bk_d = nc.declare_dram_parameter("bk", [HPC], f32, isOutput=False)
    bv_d = nc.declare_dram_parameter("bv", [HPC], f32, isOutput=False)
    mask_d = nc.declare_dram_parameter("mask", [S], f32, isOutput=False)
    out_d = nc.declare_dram_parameter("out", [S, HPC], f32, isOutput=True)

    with tile.TileContext(nc) as tc:
        import contextlib
        with contextlib.ExitStack() as stack:
            consts = stack.enter_context(tc.tile_pool(name="consts", bufs=1))
            p_qk = stack.enter_context(tc.tile_pool(name="p_qk", bufs=1))
            p_v = stack.enter_context(tc.tile_pool(name="p_v", bufs=1))
            p_e = stack.enter_context(tc.tile_pool(name="p_e", bufs=3))
            p_post = stack.enter_context(tc.tile_pool(name="p_post", bufs=1))
            p_rec = stack.enter_context(tc.tile_pool(name="p_rec", bufs=4))

            # ---- constants ----
            ident = consts.tile([128, 128], bf16)
            make_identity(nc, ident)
            ones_row = consts.tile([1, 128], bf16)
            nc.gpsimd.memset(ones_row, 1.0)
            ones6 = consts.tile([128, NHEADS], f32)
            nc.gpsimd.memset(ones6, 1.0)
            bq_sb = consts.tile([128, NPAIR], f32)
            nc.gpsimd.dma_start(out=bq_sb, in_=bq_d[:].rearrange("(t p) -> p t", p=128))
            bk_sb = consts.tile([128, NPAIR], f32)
            nc.gpsimd.dma_start(out=bk_sb, in_=bk_d[:].rearrange("(t p) -> p t", p=128))
            bv_f32 = consts.tile([1, HPC], f32)
            nc.gpsimd.dma_start(out=bv_f32, in_=bv_d[:].rearrange("(o s) -> o s", o=1))
            bv_sb = consts.tile([1, HPC], bf16)
            nc.vector.tensor_copy(bv_sb, bv_f32)
            mask_sb = consts.tile([128, NST], f32)
            nc.gpsimd.dma_start(out=mask_sb, in_=mask_d[:].rearrange("(t p) -> p t", p=128))
            wmask = consts.tile([128, NST], f32)
            nc.scalar.activation(wmask, mask_sb, AF.Exp)

            # ---- persistent ----
            qt = p_qk.tile([128, NPAIR, S], bf16)      # Q^T pairs
            kt_sb = p_qk.tile([128, NPAIR, S], bf16)   # K^T pairs
            v_sb = p_v.tile([128, NST, NHEADS, HD + 1], bf16)
            out_sb = p_post.tile([128, NST, HPC], f32)

            def build():
                with tc.tile_pool(name="p_xtw", bufs=1) as px:
                    xt = px.tile([128, NHT, S], bf16)
                    wt_q = px.tile([128, NHT, HPC], bf16)
                    wt_k = px.tile([128, NHT, HPC], bf16)
                    wt_v = px.tile([128, NHT, HPC], bf16)

                    # -- bf16 casts + transposes of x and W --
                    with tc.tile_pool(name="p_nat", bufs=1) as pn, \
                            tc.tile_pool(name="psum_util", bufs=2, space="PSUM") as pu:
                        for stq in range(4):
                            xtiles = []
                            for c in range(4):
                                st = stq * 4 + c
                                x_nat = pn.tile([128, H], f32, tag="xnat", bufs=6,
                                                name="x_nat")
                                nc.gpsimd.dma_start(out=x_nat, in_=x_d[st * 128:(st + 1) * 128, :])
                                xc = pn.tile([128, H], bf16, tag="xc", bufs=6,
                                             name="xc")
                                nc.vector.tensor_copy(xc, x_nat)
                                xtiles.append(xc)
                            for ht in range(NHT):
                                pxm = pu.tile([128, 512], bf16, tag="tp1", name="pxm")
                                for c in range(4):
                                    nc.tensor.transpose(
                                        pxm[:, c * 128:(c + 1) * 128],
                                        xtiles[c][:, ht * 128:(ht + 1) * 128],
                                        ident)
                                nc.vector.tensor_copy(xt[:, ht, stq * 512:(stq + 1) * 512], pxm)
                        for wd, wt in ((wq_d, wt_q), (wk_d, wt_k), (wv_d, wt_v)):
                            wtiles = []
                            for ot in range(3):
                                w_nat = pn.tile([128, H], f32, tag="wnat", bufs=3,
                                                name="w_nat")
                                nc.gpsimd.dma_start(out=w_nat, in_=wd[ot * 128:(ot + 1) * 128, :])
                                wc = pn.tile([128, H], bf16, tag="wc", bufs=3,
                                             name="wc")
                                nc.vector.tensor_copy(wc, w_nat)
                                wtiles.append(wc)
                            for ht in range(NHT):
                                pw = pu.tile([128, HPC], bf16, tag="tpw", name="pw",
                                             bufs=2)
                                for ot in range(3):
                                    nc.tensor.transpose(
                                        pw[:, ot * 128:(ot + 1) * 128],
                                        wtiles[ot][:, ht * 128:(ht + 1) * 128],
                                        ident)
                                nc.vector.tensor_copy(wt[:, ht, :], pw)

                    # -- QKV projections --
                    with tc.tile_pool(name="psum_qkv", bufs=1, space="PSUM") as psum_qkv:
                        for pair in range(NPAIR):
                            for wt, dst, bias in ((wt_q, qt, bq_sb), (wt_k, kt_sb, bk_sb)):
                                for sc in range(4):
                                    pq = psum_qkv.tile([128, 512], f32, tag="qt",
                                                       name="pq", bufs=2)
                                    for ht in range(NHT):
                                        nc.tensor.matmul(
                                            pq,
                                            lhsT=wt[:, ht, pair * 128:(pair + 1) * 128],
                                            rhs=xt[:, ht, sc * 512:(sc + 1) * 512],
                                            start=(ht == 0), stop=(ht == NHT - 1))
                                    nc.vector.tensor_scalar(
                                        out=dst[:, pair, sc * 512:(sc + 1) * 512],
                                        in0=pq, scalar1=bias[:, pair:pair + 1],
                                        scalar2=None, op0=OP.add)
                        for st in range(NST):
                            pv = psum_qkv.tile([128, HPC], f32, tag="v", name="pv", bufs=2)
                            for ht in range(NHT):
                                nc.tensor.matmul(
                                    pv,
                                    lhsT=xt[:, ht, st * 128:(st + 1) * 128],
                                    rhs=wt_v[:, ht, :],
                                    start=(ht == 0), stop=False)
                            nc.tensor.matmul(pv, lhsT=ones_row, rhs=bv_sb,
                                             start=False, stop=True)
                            nc.vector.tensor_scalar(
                                out=v_sb[:, st, :, 0:HD],
                                in0=pv.rearrange("p (h d) -> p h d", h=NHEADS),
                                scalar1=wmask[:, st:st + 1], scalar2=None, op0=OP.mult)
                            nc.vector.tensor_scalar(
                                out=v_sb[:, st, :, HD], in0=ones6,
                                scalar1=wmask[:, st:st + 1], scalar2=None, op0=OP.mult)

                # ---- attention: qtr outer, pair inner ----
                with tc.tile_pool(name="psum_attn", bufs=1, space="PSUM") as pa:
                    def scores_mm(dst, pair, qs, ktp, hh):
                        """scoresT for head (2*pair+hh), k-tiles 2ktp,2ktp+1."""
                        lo, hi = 64 * hh, 64 * (hh + 1)
                        for par in range(2):
                            kt = 2 * ktp + par
                            nc.tensor.matmul(
                                dst[:, par, :],
                                lhsT=kt_sb[lo:hi, pair, kt * 128:(kt + 1) * 128],
                                rhs=qt[lo:hi, pair, qs:qs + 512],
                                start=True, stop=True)

                    def ctx_mm(ctx_t, e_t, pair, ktp, hh):
                        """ctx[q,65] += E^T @ v_aug for head 2*pair+hh.

                        One PSUM bank holds all 4 j-regions; start/stop must
                        bracket the whole bank (lazy zero-region semantics),
                        so only the very first/last matmul set them."""
                        head = 2 * pair + hh
                        for par in range(2):
                            kt = 2 * ktp + par
                            for j in range(4):
                                nc.tensor.matmul(
                                    ctx_t[:, j, 0:HD + 1],
                                    lhsT=e_t[:, par, j * 128:(j + 1) * 128],
                                    rhs=v_sb[:, kt, head, :],
                                    start=(kt == 0 and j == 0),
                                    stop=(kt == NST - 1 and j == 3))

                    for qtr in range(4):
                        for pair in range(NPAIR):
                            qs = qtr * 512
                            ctx_a = pa.tile([128, 4, 128], f32, tag="ctx",
                                            bufs=4, name="ctx_a")
                            ctx_b = pa.tile([128, 4, 128], f32, tag="ctx",
                                            bufs=4, name="ctx_b")
                            prev_eb = None
                            for ktp in range(8):
                                sa = pa.tile([128, 2, 512], f32, tag="s",
                                             bufs=2, name="sa")
                                scores_mm(sa, pair, qs, ktp, 0)
                                if prev_eb is not None:
                                    ctx_mm(ctx_b, prev_eb, pair, ktp - 1, 1)
                                sb_ = pa.tile([128, 2, 512], f32, tag="s",
                                              bufs=2, name="sb_")
                                scores_mm(sb_, pair, qs, ktp, 1)
                                e_a = p_e.tile([128, 2, 512], bf16, tag="e",
                                               bufs=4, name="e_a")
                                e_b = p_e.tile([128, 2, 512], bf16, tag="e",
                                               bufs=4, name="e_b")
                                nc.scalar.activation(e_a, sa, AF.Exp,
                                                     scale=float(SCALE))
                                nc.scalar.activation(e_b, sb_, AF.Exp,
                                                     scale=float(SCALE))
                                ctx_mm(ctx_a, e_a, pair, ktp, 0)
                                prev_eb = e_b
                            ctx_mm(ctx_b, prev_eb, pair, 7, 1)

                            # epilogue: normalize in natural layout (DVE only)
                            for hh, ctx_t in ((0, ctx_a), (1, ctx_b)):
                                head = 2 * pair + hh
                                rec = p_rec.tile([128, 4, 1], f32, tag="rec",
                                                 name="rec")
                                nc.vector.reciprocal(rec, ctx_t[:, :, HD:HD + 1])
                                for j in range(4):
                                    st = qtr * 4 + j
                                    nc.vector.tensor_scalar(
                                        out=out_sb[:, st, head * HD:(head + 1) * HD],
                                        in0=ctx_t[:, j, 0:HD],
                                        scalar1=rec[:, j, :],
                                        scalar2=None, op0=OP.mult)
                        for j in range(4):
                            st = qtr * 4 + j
                            nc.gpsimd.dma_start(
                                out=out_d[st * 128:(st + 1) * 128, :],
                                in_=out_sb[:, st, :])

            if reps == 1:
                build()
            else:
                with tc.For_i(0, reps, 1):
                    build()

    nc.compile()
    return nc


def make_runner(nc):
    """jit-compiled shard_map runner over 8 cores."""
    import jax
    import numpy as _np
    from jax.sharding import Mesh, NamedSharding, PartitionSpec
    from jax.experimental.shard_map import shard_map
    import concourse.mybir as mybir
    from concourse.bass2jax import (_bass_exec_p, install_neuronx_cc_hook,
                                    partition_id_tensor)

    install_neuronx_cc_hook()
    part_name = nc.partition_id_tensor.name if nc.partition_id_tensor else None
    in_names, out_names, out_avals, out_shapes = [], [], [], []
    for alloc in nc.m.functions[0].allocations:
        if not isinstance(alloc, mybir.MemoryLocationSet):
            continue
        name = alloc.memorylocations[0].name
        if alloc.kind == "ExternalInput":
            if name != part_name:
                in_names.append(name)
        elif alloc.kind == "ExternalOutput":
            out_names.append(name)
            shape = tuple(alloc.tensor_shape)
            dtype = mybir.dt.np(alloc.dtype)
            out_avals.append(jax.core.ShapedArray(shape, dtype))
            out_shapes.append((shape, dtype))
    n_params = len(in_names)
    all_in_names = list(in_names) + list(out_names)
    if part_name is not None:
        all_in_names.append(part_name)

    def _body(*args):
        operands = list(args)
        if part_name is not None:
            operands.append(partition_id_tensor())
        outs = _bass_exec_p.bind(
            *operands,
            out_avals=tuple(out_avals),
            in_names=tuple(all_in_names),
            out_names=tuple(out_names),
            lowering_input_output_aliases=(),
            sim_require_finite=True,
            sim_require_nnan=True,
            nc=nc,
        )
        return tuple(outs)

    devices = jax.devices()[:NCORES]
    mesh = Mesh(_np.asarray(devices), ("core",))
    sharded = jax.jit(
        shard_map(_body, mesh=mesh,
                  in_specs=(PartitionSpec("core"),) * (n_params + len(out_names)),
                  out_specs=(PartitionSpec("core"),) * len(out_names),
                  check_rep=False),
        keep_unused=True)
    sh = NamedSharding(mesh, PartitionSpec("core"))

    def stage(in_maps):
        import jax as _jax
        concat_in = [
            _jax.device_put(
                _np.ascontiguousarray(
                    _np.concatenate([_np.asarray(m[name]) for m in in_maps], axis=0)),
                sh)
            for name in in_names]
        concat_zeros = [
            _jax.device_put(_np.zeros((NCORES * sh_[0], *sh_[1:]), dt), sh)
            for (sh_, dt) in out_shapes]
        return concat_in, concat_zeros

    def run(concat_in, concat_zeros):
        import jax as _jax
        outs = sharded(*concat_in, *concat_zeros)
        _jax.block_until_ready(outs)
        return outs

    def unpack(outs):
        res = []
        for c in range(NCORES):
            m = {}
            for i, name in enumerate(out_names):
                shape, dt = out_shapes[i]
                m[name] = np.asarray(outs[i]).reshape(NCORES, *shape)[c]
            res.append(m)
        return res

    return stage, run, unpack


def shard_inputs(hidden_states, attention_mask, Wq, bq, Wk, bk, Wv, bv):
    hs = np.asarray(hidden_states, dtype=np.float32)
    am = np.asarray(attention_mask, dtype=np.float32)
    Wq, Wk, Wv = (np.asarray(w, dtype=np.float32) for w in (Wq, Wk, Wv))
    bq, bk, bv = (np.asarray(b, dtype=np.float32) for b in (bq, bk, bv))
    in_maps = []
    for c in range(NCORES):
        b = c // 2
        g = c % 2
        rows = slice(g * HPC, (g + 1) * HPC)
        in_maps.append({
            "x": np.ascontiguousarray(hs[b]),
            "wq": np.ascontiguousarray(Wq[rows]),
            "wk": np.ascontiguousarray(Wk[rows]),
            "wv": np.ascontiguousarray(Wv[rows]),
            "bq": np.ascontiguousarray(bq[rows]),
            "bk": np.ascontiguousarray(bk[rows]),
            "bv": np.ascontiguousarray(bv[rows]),
            "mask": np.ascontiguousarray(am[b, 0, 0, :]),
        })
    return in_maps


def unshard_outputs(results):
    out = np.empty((B, S, H), dtype=np.float32)
    for c in range(NCORES):
        b = c // 2
        g = c % 2
        out[b][:, g * HPC:(g + 1) * HPC] = results[c]["out"]
    return out


def get_compiled(reps=1, loop_part="all"):
    key = (reps, loop_part)
    if key in _CACHE:
        return _CACHE[key]
    if True:
        nc = build_nc(reps, loop_part)
        stage, run, unpack = make_runner(nc)
        _CACHE[key] = (nc, stage, run, unpack)
    return _CACHE[key]


def kernel(hidden_states, attention_mask, Wq, bq, Wk, bk, Wv, bv):
    _, stage, run, unpack = get_compiled(reps=1)
    in_maps = shard_inputs(hidden_states, attention_mask, Wq, bq, Wk, bk, Wv, bv)
    ci, cz = stage(in_maps)
    outs = run(ci, cz)
    return unshard_outputs(unpack(outs))


# revision 4
# speedup vs baseline: 1.4565x; 1.3054x over previous
"""BertSelfAttention TRN2 kernel (v2 — ACT-engine-saturated design).

Problem: B=4, S=2048, H=768, NH=12, HD=64, fp32.
Sharding: 8 cores; core c owns batch b = c//2 and head-group g = c%2
(6 heads = 384 hidden cols). Each core runs the same Bass program on its
shard; host reassembles.

Bottleneck analysis: the softmax exp is 6*S^2 = 25.2M elements per core
and can only run on the Activation engine (1 elem/cycle/partition
@1.2GHz) => ~199us floor. Everything else is organized to hide under
that: all matmuls in bf16 (1 col/cycle on PE), context accumulated in
natural [q, d] layout with a fused denominator column (no output
transposes), and the emission order software-pipelines scores(k+1)
between the two ctx halves of k so ACT never waits on PSUM buffers.

Per-core algorithm (fp32 PSUM, bf16 operands):
  xt = x^T, wt = W^T          (PE transposes of bf16 casts)
  QT/KT[pair] = W @ xT + b    (partitions = [headA d | headB d])
  V = x @ WvT + bv, scaled by w_k = exp(mask_k); augmented col w
  per (qtr, pair, ktp):
     scoresT[k, q] = K @ QT   (2 heads x 2 k-tiles, N=512)
     E = exp(SCALE * scoresT) (ACT, [128,1024] from PSUM -> bf16 SBUF)
     ctx[q, 65] += E^T @ v_aug  (natural layout, denom in col 64)
  epilogue: rec = 1/ctx[:,64]; out = ctx[:,0:64] * rec  (DVE only)
"""

import numpy as np

B, S, H = 4, 2048, 768
NH, HD = 12, 64
SCALE = 1.0 / np.sqrt(np.float32(HD)).astype(np.float32)
HPC = H // 2          # 384 hidden cols per core (6 heads)
NHEADS = 6            # heads per core
NPAIR = 3             # head pairs per core
NST = S // 128        # 16 S-tiles
NHT = H // 128        # 6 hidden tiles
NCORES = 8

_CACHE = {}


def build_nc(reps=1, loop_part="all"):
    import concourse.bacc as bacc
    import concourse.mybir as mybir
    import concourse.tile as tile
    from concourse.masks import make_identity

    f32 = mybir.dt.float32
    bf16 = mybir.dt.bfloat16
    AF = mybir.ActivationFunctionType
    OP = mybir.AluOpType

    nc = bacc.Bacc("TRN2", target_bir_lowering=False, debug=False,
                   num_devices=NCORES)

    x_d = nc.declare_dram_parameter("x", [S, H], f32, isOutput=False)
    wq_d = nc.declare_dram_parameter("wq", [HPC, H], f32, isOutput=False)
    wk_d = nc.declare_dram_parameter("wk", [HPC, H], f32, isOutput=False)
    wv_d = nc.declare_dram_parameter("wv", [HPC, H], f32, isOutput=False)
    bq_d = nc.declare_dram_parameter("bq", [HPC], f32, isOutput=False)
    bk_d = nc.declare_dram_parameter("bk", [HPC], f32, isOutput=False)
    bv_d = nc.declare_dram_parameter("bv", [HPC], f32, isOutput=False)
    mask_d = nc.declare_dram_parameter("mask", [S], f32, isOutput=False)
    out_d = nc.declare_dram_parameter("out", [S, HPC], f32, isOutput=True)

    with tile.TileContext(nc) as tc:
        import contextlib
        with contextlib.ExitStack() as stack:
            consts = stack.enter_context(tc.tile_pool(name="consts", bufs=1))
            p_qk = stack.enter_context(tc.tile_pool(name="p_qk", bufs=1))
            p_v = stack.enter_context(tc.tile_pool(name="p_v", bufs=1))
            p_e = stack.enter_context(tc.tile_pool(name="p_e", bufs=3))
            p_post = stack.enter_context(tc.tile_pool(name="p_post", bufs=1))
            p_rec = stack.enter_context(tc.tile_pool(name="p_rec", bufs=4))

            # ---- constants ----
            ident = consts.tile([128, 128], bf16)
            make_identity(nc, ident)
            ones_row = consts.tile([1, 128], bf16)
            nc.gpsimd.memset(ones_row, 1.0)
            ones6 = consts.tile([128, NHEADS], f32)
            nc.gpsimd.memset(ones6, 1.0)
            bq_sb = consts.tile([128, NPAIR], f32)
            nc.gpsimd.dma_start(out=bq_sb, in_=bq_d[:].rearrange("(t p) -> p t", p=128))
            bk_sb = consts.tile([128, NPAIR], f32)
            nc.gpsimd.dma_start(out=bk_sb, in_=bk_d[:].rearrange("(t p) -> p t", p=128))
            bv_f32 = consts.tile([1, HPC], f32)
            nc.gpsimd.dma_start(out=bv_f32, in_=bv_d[:].rearrange("(o s) -> o s", o=1))
            bv_sb = consts.tile([1, HPC], bf16)
            nc.vector.tensor_copy(bv_sb, bv_f32)
            mask_sb = consts.tile([128, NST], f32)
            nc.gpsimd.dma_start(out=mask_sb, in_=mask_d[:].rearrange("(t p) -> p t", p=128))
            wmask = consts.tile([128, NST], f32)
            nc.scalar.activation(wmask, mask_sb, AF.Exp)

            # ---- persistent ----
            qt = p_qk.tile([128, NPAIR, S], bf16)      # Q^T pairs
            kt_sb = p_qk.tile([128, NPAIR, S], bf16)   # K^T pairs
            v_sb = p_v.tile([128, NST, NHEADS, HD + 1], bf16)
            out_sb = p_post.tile([128, NST, HPC], f32)

            def build():
                with tc.tile_pool(name="p_xw", bufs=1) as pxw, \
                        tc.tile_pool(name="p_nat", bufs=1) as pn, \
                        tc.tile_pool(name="psum_all", bufs=1, space="PSUM") as pa:
                    xt = pxw.tile([128, NHT, S], bf16)
                    wt_q = pxw.tile([128, NHT, HPC], bf16)
                    wt_k = pxw.tile([128, NHT, HPC], bf16)
                    wt_v = pxw.tile([128, NHT, HPC], bf16)

                    # ---- DMA everything up front (queues run in parallel) --
                    x_nat = []
                    for st in range(NST):
                        t = pn.tile([128, H], f32, tag="xnat", bufs=12,
                                    name="x_nat")
                        nc.gpsimd.dma_start(out=t, in_=x_d[st * 128:(st + 1) * 128, :])
                        x_nat.append(t)
                        if st == 3:
                            w_nat = {}
                            for key, wd in (("q", wq_d), ("k", wk_d), ("v", wv_d)):
                                w_nat[key] = []
                                for ot in range(3):
                                    wn = pn.tile([128, H], f32, tag="wnat",
                                                 bufs=9, name="w_nat")
                                    nc.gpsimd.dma_start(
                                        out=wn, in_=wd[ot * 128:(ot + 1) * 128, :])
                                    w_nat[key].append(wn)

                    def s_tile(name):
                        return pa.tile([128, 2, 512], f32, tag="s", bufs=3,
                                       name=name)

                    xc_tiles = {}

                    def xt_item(stq, ht):
                        """Transpose x chunk stq, hidden tile ht -> xt."""
                        if ht == 0:
                            xc_tiles[stq] = []
                            for c in range(4):
                                xc = pn.tile([128, H], bf16, tag="xc", bufs=8,
                                             name="xc")
                                nc.vector.tensor_copy(xc, x_nat[stq * 4 + c])
                                xc_tiles[stq].append(xc)
                        ps = s_tile("tp_x").bitcast(bf16)
                        for c in range(4):
                            nc.tensor.matmul(
                                ps[:, 0, c * 128:(c + 1) * 128],
                                lhsT=xc_tiles[stq][c][:, ht * 128:(ht + 1) * 128],
                                rhs=ident, is_transpose=True,
                                start=(c == 0), stop=(c == 3))
                        nc.vector.tensor_copy(
                            xt[:, ht, stq * 512:(stq + 1) * 512], ps[:, 0, 0:512])

                    wc_tiles = {}

                    def wt_item(key, ht):
                        wt = {"q": wt_q, "k": wt_k, "v": wt_v}[key]
                        if ht == 0:
                            wc_tiles[key] = []
                            for ot in range(3):
                                wc = pn.tile([128, H], bf16, tag="wc", bufs=4,
                                             name="wc")
                                nc.vector.tensor_copy(wc, w_nat[key][ot])
                                wc_tiles[key].append(wc)
                        ps = s_tile("tp_w").bitcast(bf16)
                        for ot in range(3):
                            nc.tensor.matmul(
                                ps[:, 0, ot * 128:(ot + 1) * 128],
                                lhsT=wc_tiles[key][ot][:, ht * 128:(ht + 1) * 128],
                                rhs=ident, is_transpose=True,
                                start=(ot == 0), stop=(ot == 2))
                        nc.vector.tensor_copy(wt[:, ht, :], ps[:, 0, 0:HPC])

                    def qk_item(key, pair, sc):
                        """Project one [128,512] chunk of Q^T or K^T."""
                        wt, dst, bias = {
                            "q": (wt_q, qt, bq_sb),
                            "k": (wt_k, kt_sb, bk_sb)}[key]
                        ps = s_tile("pq")
                        pq = ps[:, 0, :]
                        for ht in range(NHT):
                            nc.tensor.matmul(
                                pq,
                                lhsT=wt[:, ht, pair * 128:(pair + 1) * 128],
                                rhs=xt[:, ht, sc * 512:(sc + 1) * 512],
                                start=(ht == 0), stop=(ht == NHT - 1))
                        nc.vector.tensor_scalar(
                            out=dst[:, pair, sc * 512:(sc + 1) * 512],
                            in0=pq, scalar1=bias[:, pair:pair + 1],
                            scalar2=None, op0=OP.add)

                    def v_item(st):
                        ps = s_tile("pv")
                        pv = ps[:, 0, 0:HPC]
                        for ht in range(NHT):
                            nc.tensor.matmul(
                                pv,
                                lhsT=xt[:, ht, st * 128:(st + 1) * 128],
                                rhs=wt_v[:, ht, :],
                                start=(ht == 0), stop=False)
                        nc.tensor.matmul(pv, lhsT=ones_row, rhs=bv_sb,
                                         start=False, stop=True)
                        nc.vector.tensor_scalar(
                            out=v_sb[:, st, :, 0:HD],
                            in0=pv.rearrange("p (h d) -> p h d", h=NHEADS),
                            scalar1=wmask[:, st:st + 1], scalar2=None, op0=OP.mult)
                        nc.vector.tensor_scalar(
                            out=v_sb[:, st, :, HD], in0=ones6,
                            scalar1=wmask[:, st:st + 1], scalar2=None, op0=OP.mult)

                    # ---- prelude: minimum work before attention can start --
                    for ht in range(NHT):
                        wt_item("q", ht)
                    for ht in range(NHT):
                        wt_item("k", ht)
                    for ht in range(NHT):
                        xt_item(0, ht)
                    qk_item("k", 0, 0)
                    qk_item("q", 0, 0)
                    for ht in range(NHT):
                        wt_item("v", ht)
                    v_item(0)
                    v_item(1)

                    # ---- deferred work, injected into attention slots ------
                    # inject[unit][ktp] -> list of closures; deadlines:
                    #   k-chunk sc by end of ktp 2sc-1, v st by ktp st//2,
                    #   q-chunk qtr(pair0) before unit qtr, pair p by unit 4p.
                    def I(fn, *a):
                        return lambda: fn(*a)

                    inject = {
                        (0, 0): [I(v_item, 2), I(v_item, 3)] +
                                [I(xt_item, 1, h) for h in range(3)],
                        (0, 1): [I(xt_item, 1, h) for h in range(3, 6)] +
                                [I(qk_item, "k", 0, 1)],
                        (0, 2): [I(v_item, 4), I(v_item, 5)] +
                                [I(xt_item, 2, h) for h in range(4)],
                        (0, 3): [I(xt_item, 2, h) for h in range(4, 6)] +
                                [I(qk_item, "k", 0, 2), I(v_item, 6), I(v_item, 7)],
                        (0, 4): [I(v_item, 8), I(v_item, 9)] +
                                [I(xt_item, 3, h) for h in range(4)],
                        (0, 5): [I(xt_item, 3, h) for h in range(4, 6)] +
                                [I(qk_item, "k", 0, 3), I(v_item, 10), I(v_item, 11)],
                        (0, 6): [I(v_item, 12), I(v_item, 13), I(qk_item, "q", 0, 1)],
                        (0, 7): [I(v_item, 14), I(v_item, 15)],
                        (1, 0): [I(qk_item, "q", 0, 2)],
                        (1, 2): [I(qk_item, "q", 0, 3)],
                        (1, 4): [I(qk_item, "k", 1, 0)],
                        (1, 6): [I(qk_item, "k", 1, 1)],
                        (2, 0): [I(qk_item, "k", 1, 2)],
                        (2, 2): [I(qk_item, "k", 1, 3)],
                        (2, 4): [I(qk_item, "q", 1, 0)],
                        (2, 6): [I(qk_item, "q", 1, 1)],
                        (3, 0): [I(qk_item, "q", 1, 2)],
                        (3, 2): [I(qk_item, "q", 1, 3)],
                        (3, 4): [I(qk_item, "k", 2, 0)],
                        (3, 6): [I(qk_item, "k", 2, 1)],
                        (4, 0): [I(qk_item, "k", 2, 2)],
                        (4, 2): [I(qk_item, "k", 2, 3)],
                        (4, 4): [I(qk_item, "q", 2, 0)],
                        (4, 6): [I(qk_item, "q", 2, 1)],
                        (5, 0): [I(qk_item, "q", 2, 2)],
                        (5, 2): [I(qk_item, "q", 2, 3)],
                    }

                    # ---- attention: pair outer, qtr inner ------------------
                    def scores_mm(dst, pair, qs, ktp, hh):
                        """scoresT for head (2*pair+hh), k-tiles 2ktp,2ktp+1."""
                        lo, hi = 64 * hh, 64 * (hh + 1)
                        for par in range(2):
                            kt = 2 * ktp + par
                            nc.tensor.matmul(
                                dst[:, par, :],
                                lhsT=kt_sb[lo:hi, pair, kt * 128:(kt + 1) * 128],
                                rhs=qt[lo:hi, pair, qs:qs + 512],
                                start=True, stop=True)

                    def ctx_mm(ctx_t, e_t, pair, ktp, hh):
                        """ctx[q,65] += E^T @ v_aug for head 2*pair+hh.

                        One PSUM bank holds all 4 j-regions; start/stop must
                        bracket the whole bank (lazy zero-region semantics),
                        so only the very first/last matmul set them."""
                        head = 2 * pair + hh
                        for par in range(2):
                            kt = 2 * ktp + par
                            for j in range(4):
                                nc.tensor.matmul(
                                    ctx_t[:, j, 0:HD + 1],
                                    lhsT=e_t[:, par, j * 128:(j + 1) * 128],
                                    rhs=v_sb[:, kt, head, :],
                                    start=(kt == 0 and j == 0),
                                    stop=(kt == NST - 1 and j == 3))

                    for pair in range(NPAIR):
                        for qtr in range(4):
                            unit = pair * 4 + qtr
                            qs = qtr * 512
                            ctx_a = pa.tile([128, 4, 128], f32, tag="ctx",
                                            bufs=2, name="ctx_a")
                            ctx_b = pa.tile([128, 4, 128], f32, tag="ctx",
                                            bufs=2, name="ctx_b")
                            prev_eb = None
                            for ktp in range(8):
                                sa = s_tile("sa")
                                scores_mm(sa, pair, qs, ktp, 0)
                                if prev_eb is not None:
                                    ctx_mm(ctx_b, prev_eb, pair, ktp - 1, 1)
                                sb_ = s_tile("sb_")
                                scores_mm(sb_, pair, qs, ktp, 1)
                                e_a = p_e.tile([128, 2, 512], bf16, tag="e",
                                               bufs=4, name="e_a")
                                e_b = p_e.tile([128, 2, 512], bf16, tag="e",
                                               bufs=4, name="e_b")
                                nc.scalar.activation(e_a, sa, AF.Exp,
                                                     scale=float(SCALE))
                                nc.scalar.activation(e_b, sb_, AF.Exp,
                                                     scale=float(SCALE))
                                for item in inject.pop((unit, ktp), ()):
                                    item()
                                ctx_mm(ctx_a, e_a, pair, ktp, 0)
                                prev_eb = e_b
                            ctx_mm(ctx_b, prev_eb, pair, 7, 1)

                            # epilogue: normalize in natural layout (DVE only)
                            for hh, ctx_t in ((0, ctx_a), (1, ctx_b)):
                                head = 2 * pair + hh
                                rec = p_rec.tile([128, 4, 1], f32, tag="rec",
                                                 name="rec")
                                nc.vector.reciprocal(rec, ctx_t[:, :, HD:HD + 1])
                                for j in range(4):
                                    st = qtr * 4 + j
                                    nc.vector.tensor_scalar(
                                        out=out_sb[:, st, head * HD:(head + 1) * HD],
                                        in0=ctx_t[:, j, 0:HD],
                                        scalar1=rec[:, j, :],
                                        scalar2=None, op0=OP.mult)
                            if pair == NPAIR - 1:
                                for j in range(4):
                                    st = qtr * 4 + j
                                    nc.gpsimd.dma_start(
                                        out=out_d[st * 128:(st + 1) * 128, :],
                                        in_=out_sb[:, st, :])
                    assert not inject, f"unconsumed inject items: {list(inject)}"

            if reps == 1:
                build()
            else:
                with tc.For_i(0, reps, 1):
                    build()

    nc.compile()
    return nc


def make_runner(nc):
    """jit-compiled shard_map runner over 8 cores."""
    import jax
    import numpy as _np
    from jax.sharding import Mesh, NamedSharding, PartitionSpec
    from jax.experimental.shard_map import shard_map
    import concourse.mybir as mybir
    from concourse.bass2jax import (_bass_exec_p, install_neuronx_cc_hook,
                                    partition_id_tensor)

    install_neuronx_cc_hook()
    part_name = nc.partition_id_tensor.name if nc.partition_id_tensor else None
    in_names, out_names, out_avals, out_shapes = [], [], [], []
    for alloc in nc.m.functions[0].allocations:
        if not isinstance(alloc, mybir.MemoryLocationSet):
            continue
        name = alloc.memorylocations[0].name
        if alloc.kind == "ExternalInput":
            if name != part_name:
                in_names.append(name)
        elif alloc.kind == "ExternalOutput":
            out_names.append(name)
            shape = tuple(alloc.tensor_shape)
            dtype = mybir.dt.np(alloc.dtype)
            out_avals.append(jax.core.ShapedArray(shape, dtype))
            out_shapes.append((shape, dtype))
    n_params = len(in_names)
    all_in_names = list(in_names) + list(out_names)
    if part_name is not None:
        all_in_names.append(part_name)

    def _body(*args):
        operands = list(args)
        if part_name is not None:
            operands.append(partition_id_tensor())
        outs = _bass_exec_p.bind(
            *operands,
            out_avals=tuple(out_avals),
            in_names=tuple(all_in_names),
            out_names=tuple(out_names),
            lowering_input_output_aliases=(),
            sim_require_finite=True,
            sim_require_nnan=True,
            nc=nc,
        )
        return tuple(outs)

    devices = jax.devices()[:NCORES]
    mesh = Mesh(_np.asarray(devices), ("core",))
    sharded = jax.jit(
        shard_map(_body, mesh=mesh,
                  in_specs=(PartitionSpec("core"),) * (n_params + len(out_names)),
                  out_specs=(PartitionSpec("core"),) * len(out_names),
                  check_rep=False),
        keep_unused=True)
    sh = NamedSharding(mesh, PartitionSpec("core"))

    def stage(in_maps):
        import jax as _jax
        concat_in = [
            _jax.device_put(
                _np.ascontiguousarray(
                    _np.concatenate([_np.asarray(m[name]) for m in in_maps], axis=0)),
                sh)
            for name in in_names]
        concat_zeros = [
            _jax.device_put(_np.zeros((NCORES * sh_[0], *sh_[1:]), dt), sh)
            for (sh_, dt) in out_shapes]
        return concat_in, concat_zeros

    def run(concat_in, concat_zeros):
        import jax as _jax
        outs = sharded(*concat_in, *concat_zeros)
        _jax.block_until_ready(outs)
        return outs

    def unpack(outs):
        res = []
        for c in range(NCORES):
            m = {}
            for i, name in enumerate(out_names):
                shape, dt = out_shapes[i]
                m[name] = np.asarray(outs[i]).reshape(NCORES, *shape)[c]
            res.append(m)
        return res

    return stage, run, unpack


def shard_inputs(hidden_states, attention_mask, Wq, bq, Wk, bk, Wv, bv):
    hs = np.asarray(hidden_states, dtype=np.float32)
    am = np.asarray(attention_mask, dtype=np.float32)
    Wq, Wk, Wv = (np.asarray(w, dtype=np.float32) for w in (Wq, Wk, Wv))
    bq, bk, bv = (np.asarray(b, dtype=np.float32) for b in (bq, bk, bv))
    in_maps = []
    for c in range(NCORES):
        b = c // 2
        g = c % 2
        rows = slice(g * HPC, (g + 1) * HPC)
        in_maps.append({
            "x": np.ascontiguousarray(hs[b]),
            "wq": np.ascontiguousarray(Wq[rows]),
            "wk": np.ascontiguousarray(Wk[rows]),
            "wv": np.ascontiguousarray(Wv[rows]),
            "bq": np.ascontiguousarray(bq[rows]),
            "bk": np.ascontiguousarray(bk[rows]),
            "bv": np.ascontiguousarray(bv[rows]),
            "mask": np.ascontiguousarray(am[b, 0, 0, :]),
        })
    return in_maps


def unshard_outputs(results):
    out = np.empty((B, S, H), dtype=np.float32)
    for c in range(NCORES):
        b = c // 2
        g = c % 2
        out[b][:, g * HPC:(g + 1) * HPC] = results[c]["out"]
    return out


def get_compiled(reps=1, loop_part="all"):
    key = (reps, loop_part)
    if key in _CACHE:
        return _CACHE[key]
    if True:
        nc = build_nc(reps, loop_part)
        stage, run, unpack = make_runner(nc)
        _CACHE[key] = (nc, stage, run, unpack)
    return _CACHE[key]


def kernel(hidden_states, attention_mask, Wq, bq, Wk, bk, Wv, bv):
    _, stage, run, unpack = get_compiled(reps=1)
    in_maps = shard_inputs(hidden_states, attention_mask, Wq, bq, Wk, bk, Wv, bv)
    ci, cz = stage(in_maps)
    outs = run(ci, cz)
    return unshard_outputs(unpack(outs))
